# revision 32
# baseline (speedup 1.0000x reference)
"""HorNet-style block (gnconv + MLP) on 8 TRN2 NeuronCores.

Data-parallel over batch: 16 images -> 2 per core. Each core runs the full
block on its shard; no collectives. Layout: channels on partitions, spatial
(b, h, w) on the free axis. Matmul operands in bf16 (layer-scale gammas are
1e-6, so branch precision is uncritical); PSUM/residual path in f32.

Depthwise 7x7 conv: column-tiled matmuls. For each 128-channel chunk, the
4 channel-groups of 32 are processed by 4 concurrent col-tiled matmuls
(tile_position via output partition offset). The moving operand T_g holds 4
row-shifted copies of group g on the partition axis, so each matmul covers
4 dy-taps at once through the K axis: 2 dy-bands x 7 dx = 14 matmul rounds
instead of 49. Weight transposes go through a bf16 DRAM round-trip + XBAR
DMA-transpose with multi-buffered staging to keep PE busy from the start.
"""

import numpy as np

P = 128
DIM = 512
ORDER = 5
DIMS = [32, 64, 128, 256, 512]
DW = 992
EPS = 1e-6
BC = 2          # images per core
NCORES = 8
HID = 4 * DIM   # 2048

NB = [(b, ht) for b in range(BC) for ht in range(2)]  # 4 blocks of [16 rows x 32]

_CACHE = {}


def _build():
    import concourse.mybir as mybir
    import concourse.tile as tile
    from concourse import bacc
    from concourse.masks import make_identity

    F32 = mybir.dt.float32
    BF16 = mybir.dt.bfloat16
    AL = mybir.AluOpType
    AF = mybir.ActivationFunctionType

    nc = bacc.Bacc("TRN2", target_bir_lowering=False, debug=False,
                   num_devices=NCORES)

    # ---- DRAM tensors (per-core shapes) ----
    x_d = nc.dram_tensor("x", [BC, DIM, 32, 32], F32, kind="ExternalInput").ap()
    ln1_w = nc.dram_tensor("ln1_w", [DIM], F32, kind="ExternalInput").ap()
    ln1_b = nc.dram_tensor("ln1_b", [DIM], F32, kind="ExternalInput").ap()
    pin_w = nc.dram_tensor("pin_w", [2 * DIM, DIM], F32, kind="ExternalInput").ap()
    pin_b = nc.dram_tensor("pin_b", [2 * DIM], F32, kind="ExternalInput").ap()
    dw_w = nc.dram_tensor("dw_w", [DW, 1, 7, 7], F32, kind="ExternalInput").ap()
    dw_b = nc.dram_tensor("dw_b", [DW], F32, kind="ExternalInput").ap()
    pw_w = []
    pw_b = []
    for i in range(ORDER - 1):
        pw_w.append(nc.dram_tensor(f"pw{i}_w", [DIMS[i + 1], DIMS[i]], F32,
                                   kind="ExternalInput").ap())
        pw_b.append(nc.dram_tensor(f"pw{i}_b", [DIMS[i + 1]], F32,
                                   kind="ExternalInput").ap())
    pout_w = nc.dram_tensor("pout_w", [DIM, DIM], F32, kind="ExternalInput").ap()
    pout_b = nc.dram_tensor("pout_b", [DIM], F32, kind="ExternalInput").ap()
    ln2_w = nc.dram_tensor("ln2_w", [DIM], F32, kind="ExternalInput").ap()
    ln2_b = nc.dram_tensor("ln2_b", [DIM], F32, kind="ExternalInput").ap()
    fc1_w = nc.dram_tensor("fc1_w", [HID, DIM], F32, kind="ExternalInput").ap()
    fc1_b = nc.dram_tensor("fc1_b", [HID], F32, kind="ExternalInput").ap()
    fc2_w = nc.dram_tensor("fc2_w", [DIM, HID], F32, kind="ExternalInput").ap()
    fc2_b = nc.dram_tensor("fc2_b", [DIM], F32, kind="ExternalInput").ap()
    g1_d = nc.dram_tensor("g1", [DIM], F32, kind="ExternalInput").ap()
    g2_d = nc.dram_tensor("g2", [DIM], F32, kind="ExternalInput").ap()
    out_d = nc.dram_tensor("out", [BC, DIM, 32, 32], F32, kind="ExternalOutput").ap()

    x_cf = x_d.rearrange("b c h w -> c b h w")     # [512, 2, 32, 32]
    out_cf = out_d.rearrange("b c h w -> c b h w")
    dw_flat = dw_w.rearrange("c one kh kw -> c (one kh kw)")   # [992, 49]

    with tile.TileContext(nc) as tc:
        def T(pool, shape, dtype, tag, bufs=None):
            return pool.tile(shape, dtype, tag=tag, name=tag, bufs=bufs)

        def dma(dst, src):
            nc.sync.dma_start(dst, src)

        # ------- long-lived pools -------
        cst = tc.alloc_tile_pool(name="cst", bufs=1, side="left")
        tp = tc.alloc_tile_pool(name="tp", bufs=2, side="left")
        wPin = tc.alloc_tile_pool(name="wPin", bufs=1, side="left")
        wPw = tc.alloc_tile_pool(name="wPw", bufs=1, side="left")
        drp = tc.alloc_tile_pool(name="drp", bufs=2, side="left", space="DRAM")

        def col(dram_vec, off, sz, tag):
            t = T(cst, [sz, 1], F32, tag=tag)
            dma(t[:, :], dram_vec[off:off + sz].rearrange("(p o) -> p o", o=1))
            return t

        # ---- constants ----
        ident = T(cst, [P, P], F32, tag="ident")
        make_identity(nc, ident[:])
        ones = T(cst, [P, 1], BF16, tag="ones")
        nc.gpsimd.memset(ones[:], 1.0)
        eps_c = T(cst, [1, 1], F32, tag="eps_c")
        nc.gpsimd.memset(eps_c[:], EPS)
        # I4: 4 stacked diag32 blocks; I4[32i+c, m] = (c == m)
        i4 = T(cst, [P, 32], F32, tag="i4")
        for i in range(4):
            dma(i4[32 * i:32 * (i + 1), :], ident[0:32, 0:32])

        # ---- dw weight columns, built on the PE up front ----
        # per (chunk, group, band): [4*32, 7] / [3*32, 7] tile with partition
        # 32i+c = w[ch_base + c, ky0 + i, kx]. Load a [128, 49] block per
        # chunk and replicate each 32-channel group to the 4 partition strips
        # with an identity-slice stationary (col-tiled), selecting the ky row
        # range by slicing the moving operand. Avoids thousands of tiny
        # gather DMAs on the DGE ring.
        identb = T(cst, [P, P], BF16, tag="identb")
        nc.vector.tensor_copy(identb[:], ident[:])
        wcolA = [[None] * 4 for _ in range(8)]
        wcolB = [[None] * 4 for _ in range(8)]
        pV = tc.alloc_tile_pool(name="pV", bufs=2, space="PSUM")
        dwp = tc.alloc_tile_pool(name="dwp", bufs=1, side="left")
        for q in range(8):
            dww = T(dwp, [P, 49], F32, tag="dww", bufs=2)
            if q == 0:
                nc.gpsimd.memset(dww[0:32, :], 0.0)
                dma(dww[32:128, :], dw_flat[0:96])
            else:
                dma(dww[:], dw_flat[96 + 128 * (q - 1):96 + 128 * q])
            dwwb = T(dwp, [P, 49], BF16, tag="dwwb", bufs=2)
            nc.gpsimd.tensor_copy(dwwb[:], dww[:])
            for g in range(4):
                if q == 0 and g == 0:
                    continue  # pwa channels, no dw conv
                vps = T(pV, [P, 14], F32, tag="vps")
                for i in range(4):
                    nc.tensor.matmul(vps[32 * i:32 * (i + 1), 0:7],
                                     identb[:, 32 * g:32 * (g + 1)],
                                     dwwb[:, 7 * i:7 * i + 7],
                                     start=True, stop=True,
                                     skip_group_check=True,
                                     tile_position=(0, 32 * i))
                for i in range(3):
                    nc.tensor.matmul(vps[32 * i:32 * (i + 1), 7:14],
                                     identb[:, 32 * g:32 * (g + 1)],
                                     dwwb[:, 7 * (4 + i):7 * (4 + i) + 7],
                                     start=True, stop=True,
                                     skip_group_check=True,
                                     tile_position=(0, 32 * i))
                wa = T(cst, [P, 7], F32, tag=f"wcA{q}_{g}")
                nc.scalar.copy(wa[:], vps[:, 0:7])
                wb = T(cst, [96, 7], F32, tag=f"wcB{q}_{g}")
                nc.scalar.copy(wb[:], vps[0:96, 7:14])
                wcolA[q][g] = wa
                wcolB[q][g] = wb

        # ---- load x (right stack; released after xn) ----
        xnp = tc.alloc_tile_pool(name="xnp", bufs=1, side="right")
        wnatA = tc.alloc_tile_pool(name="wnatA", bufs=1, side="right")
        xr = tc.alloc_tile_pool(name="xr", bufs=1, side="right")
        x_t = []
        for c in range(4):
            t = T(xr, [P, BC, 32, 32], F32, tag=f"x{c}")
            dma(t[:], x_cf[c * P:(c + 1) * P])
            x_t.append(t)

        # ---- channels-first LayerNorm stats -> broadcast (u_b, r_b) ----
        # sequential 4-matmul accumulation groups through a 2-bank rotation,
        # so only 2 PSUM banks are used (coexists with other PSUM pools)
        def layernorm_bcast(lnp, pS, src_tiles):
            u_row = T(lnp, [1, BC, 32, 32], F32, tag="u_row")
            s2_row = T(lnp, [1, BC, 32, 32], F32, tag="s2_row")
            sd_row = T(lnp, [1, BC, 32, 32], F32, tag="sd_row")
            xbf = []
            sqf = []
            for c in range(4):
                xb = T(lnp, [P, BC, 32, 32], BF16, tag=f"xbf{c}")
                nc.vector.tensor_copy(xb[:], src_tiles[c][:])
                xbf.append(xb)
                sq = T(lnp, [P, BC, 32, 32], BF16, tag=f"sqf{c}")
                nc.scalar.square(sq[:], src_tiles[c][:])
                sqf.append(sq)
            # 8 stat sums as 2 rounds of 4 col-tiled concurrent matmul groups
            for half in range(2):
                ps = T(pS, [P, 16, 32], F32, tag="stps", bufs=2)
                grp = []
                for j in range(4):
                    gi = half * 4 + j
                    nb, stat = gi // 2, gi % 2
                    b, ht = NB[nb]
                    grp.append((j, b, ht * 16, xbf if stat == 0 else sqf,
                                u_row if stat == 0 else s2_row))
                for c in range(4):
                    for (j, b, r0, srcs, _) in grp:
                        nc.tensor.matmul(ps[32 * j:32 * j + 1, :, :],
                                         ones[:],
                                         srcs[c][:, b, r0:r0 + 16, :],
                                         start=(c == 0), stop=(c == 3),
                                         skip_group_check=True,
                                         tile_position=(0, 32 * j))
                for (j, b, r0, _, dst) in grp:
                    nc.scalar.mul(dst[:, b, r0:r0 + 16, :],
                                  ps[32 * j:32 * j + 1, :, :], 1.0 / DIM)
            nc.vector.tensor_mul(sd_row[:], u_row[:], u_row[:])
            nc.vector.tensor_sub(s2_row[:], s2_row[:], sd_row[:])
            # r = 1/sqrt(v+eps) as exp(-0.5*ln(v+eps)): two fast scalar LUT
            # ops instead of the 8-cycle/elem DVE reciprocal (branch-only
            # precision, suppressed by the 1e-6 layer scale)
            nc.scalar.activation(sd_row[:], s2_row[:], AF.Ln, bias=eps_c[:])
            nc.scalar.activation(s2_row[:], sd_row[:], AF.Exp, scale=-0.5)
            u16 = T(lnp, [1, BC, 32, 32], BF16, tag="u16")
            r16 = T(lnp, [1, BC, 32, 32], BF16, tag="r16")
            nc.scalar.copy(u16[:], u_row[:])
            nc.scalar.copy(r16[:], s2_row[:])
            u_b = T(lnp, [P, BC, 32, 32], BF16, tag="u_b")
            r_b = T(lnp, [P, BC, 32, 32], BF16, tag="r_b")
            nc.gpsimd.partition_broadcast(u_b[:], u16[:])
            nc.gpsimd.partition_broadcast(r_b[:], r16[:])
            return u_b, r_b, xbf

        # ---- ALL weight transposes on the PE, up front ----
        # Staging loads are batched [128, 2, 512] (few DMA instructions, big
        # descriptors); transposes run on the otherwise-idle early PE with
        # 4-deep PSUM rotation; evacuation (with the f32->bf16 cast for
        # free) alternates scalar/vector. No DRAM round-trip, no XBAR.
        pT = tc.alloc_tile_pool(name="pT", bufs=4, space="PSUM")
        evq = [0]

        def evac_t(dst, srcp):
            if evq[0] % 2 == 0:
                nc.scalar.copy(dst, srcp)
            else:
                nc.vector.tensor_copy(dst, srcp)
            evq[0] += 1

        def pe_transpose(dst_tiles, dram_w, rows, cols_n):
            if rows < P:
                nat = T(wnatA, [P, 2, 512], F32, tag="wnatS", bufs=2)
                dma(nat[:rows, 0, 0:cols_n], dram_w[:, :])
                for k0 in range(0, cols_n, P):
                    ksz = min(P, cols_n - k0)
                    ps = T(pT, [P, P], F32, tag="tps")
                    nc.tensor.transpose(ps[:ksz, 0:rows],
                                        nat[:rows, 0, k0:k0 + ksz],
                                        ident[:rows, 0:rows])
                    evac_t(dst_tiles[k0 // P][:ksz, 0:rows],
                           ps[:ksz, 0:rows])
                return
            wre = dram_w.rearrange("(s p) c -> p s c", p=P)
            S = rows // P
            for s0 in range(0, S, 2):
                sn = min(2, S - s0)
                for c0 in range(0, cols_n, 512):
                    csz = min(512, cols_n - c0)
                    nat = T(wnatA, [P, 2, 512], F32, tag="wnatS", bufs=2)
                    dma(nat[:, 0:sn, 0:csz], wre[:, s0:s0 + sn, c0:c0 + csz])
                    for j in range(sn):
                        r0 = (s0 + j) * P
                        for k0 in range(0, csz, P):
                            ksz = min(P, csz - k0)
                            ps = T(pT, [P, P], F32, tag="tps")
                            nc.tensor.transpose(ps[:ksz, :],
                                                nat[:, j, k0:k0 + ksz],
                                                ident[:])
                            evac_t(dst_tiles[(c0 + k0) // P][:ksz, r0:r0 + P],
                                   ps[:ksz, :])

        pinT = [T(wPin, [P, 2 * DIM], BF16, tag=f"pinT{c}") for c in range(4)]
        pe_transpose(pinT, pin_w, 2 * DIM, DIM)

        # fc1/fc2 transposes via bf16 DRAM round-trip + XBAR, issued at t=0:
        # their ~100us latency chain hides entirely under the prologue, and
        # they stay off the PE (which transposes the smaller weights).
        cast_state = [0]

        def cast_op(dst, srcv):
            i = cast_state[0] % 2
            cast_state[0] += 1
            (nc.gpsimd, nc.vector)[i].tensor_copy(dst, srcv)

        def transpose_w(wnp, dram_w, rows, cols_n, dst_tiles):
            wsc = T(drp, [rows, cols_n], BF16, tag="wsc")
            for r0 in range(0, rows, P):
                rsz = min(P, rows - r0)
                for c0 in range(0, cols_n, 512):
                    csz = min(512, cols_n - c0)
                    nat = T(wnp, [rsz, csz], F32, tag="wnat", bufs=2)
                    dma(nat[:], dram_w[r0:r0 + rsz, c0:c0 + csz])
                    natb = T(wnp, [rsz, csz], BF16, tag="wnatb", bufs=2)
                    cast_op(natb[:], nat[:])
                    dma(wsc[r0:r0 + rsz, c0:c0 + csz], natb[:])
            for k0 in range(0, cols_n, P):
                ksz = min(P, cols_n - k0)
                nc.sync.dma_start_transpose(dst_tiles[k0 // P][:ksz, 0:rows],
                                            wsc[:, k0:k0 + ksz])

        fc1T = [T(wPw, [P, HID], BF16, tag=f"fc1T{c}") for c in range(4)]
        transpose_w(wnatA, fc1_w, HID, DIM, fc1T)
        fc2T = [T(wPw, [P, DIM], BF16, tag=f"fc2T{qq}") for qq in range(16)]
        transpose_w(wnatA, fc2_w, DIM, HID, fc2T)

        lnp1 = tc.alloc_tile_pool(name="lnp1", bufs=1, side="right")
        pS1 = tc.alloc_tile_pool(name="pS1", bufs=1, space="PSUM")
        u1b, r1b, xbf1 = layernorm_bcast(lnp1, pS1, x_t)
        xn = []
        for c in range(4):
            t = T(xnp, [P, BC, 32, 32], BF16, tag=f"xn{c}")
            nc.vector.tensor_sub(t[:], xbf1[c][:], u1b[:])
            nc.vector.tensor_mul(t[:], t[:], r1b[:])
            xn.append(t)
        pS1.release()

        # remaining weight transposes (gate-chain, proj_out, MLP)
        pw0T = T(wPw, [P, 64], BF16, tag="pw0T")
        nc.gpsimd.memset(pw0T[:], 0.0)
        pe_transpose([pw0T], pw_w[0], 64, 32)
        pw1T = T(wPw, [P, P], BF16, tag="pw1T")
        nc.gpsimd.memset(pw1T[:], 0.0)
        pe_transpose([pw1T], pw_w[1], 128, 64)
        pw2T = T(wPw, [P, 256], BF16, tag="pw2T")
        pe_transpose([pw2T], pw_w[2], 256, 128)
        pw3T = [T(wPw, [P, DIM], BF16, tag=f"pw3T{c}") for c in range(2)]
        pe_transpose(pw3T, pw_w[3], DIM, 256)
        poutT = [T(wPw, [P, DIM], BF16, tag=f"poutT{c}") for c in range(4)]
        pe_transpose(poutT, pout_w, DIM, DIM)

        pT.release()
        pV.release()
        lnp1.release()
        xr.release()
        wnatA.release()

        # ---- bias / scale columns ----
        ln1w_c = [col(ln1_w, c * P, P, f"ln1w{c}") for c in range(4)]
        ln2w_c = [col(ln2_w, c * P, P, f"ln2w{c}") for c in range(4)]
        g1_c = [col(g1_d, c * P, P, f"g1{c}") for c in range(4)]
        g2_c = [col(g2_d, c * P, P, f"g2{c}") for c in range(4)]
        poutb_c = [col(pout_b, c * P, P, f"poutb{c}") for c in range(4)]
        fc2b_c = [col(fc2_b, c * P, P, f"fc2b{c}") for c in range(4)]
        ln1b_c = [col(ln1_b, c * P, P, f"ln1b{c}") for c in range(4)]
        ln2b_c = [col(ln2_b, c * P, P, f"ln2b{c}") for c in range(4)]
        ln1b_cb = []
        ln2b_cb = []
        for c in range(4):
            tb = T(cst, [P, 1], BF16, tag=f"ln1bb{c}")
            nc.gpsimd.tensor_copy(tb[:], ln1b_c[c][:])
            ln1b_cb.append(tb)
            tb2 = T(cst, [P, 1], BF16, tag=f"ln2bb{c}")
            nc.gpsimd.tensor_copy(tb2[:], ln2b_c[c][:])
            ln2b_cb.append(tb2)
        bg1_c = []
        bg2_c = []
        for c in range(4):
            t = T(cst, [P, 1], F32, tag=f"bg1{c}")
            nc.vector.tensor_mul(t[:], poutb_c[c][:], g1_c[c][:])
            bg1_c.append(t)
            t2 = T(cst, [P, 1], F32, tag=f"bg2{c}")
            nc.vector.tensor_mul(t2[:], fc2b_c[c][:], g2_c[c][:])
            bg2_c.append(t2)
        pwb_c = []
        for i in range(ORDER - 1):
            sz = DIMS[i + 1]
            pwb_c.append([col(pw_b[i], k, min(P, sz - k), f"pwb{i}_{k}")
                          for k in range(0, sz, P)])

        # dw bias columns; chunk q>=1 = fused 128q..+128, chunk0 split base-0
        dwb_c = [None]
        for q in range(1, 8):
            dwb_c.append(col(dw_b, 96 + 128 * (q - 1), P, f"dwb{q}"))
        dwb00 = col(dw_b, 0, 32, "dwb00")
        dwb01 = col(dw_b, 32, 64, "dwb01")

        # ---- pin bias fold: pinb' = pin_b + pin_w @ ln1_b (pre ln1_w scale)
        pinb_raw = [col(pin_b, mi * P, P, f"pinb{mi}") for mi in range(8)]
        pB = tc.alloc_tile_pool(name="pB", bufs=2, space="PSUM")
        pinb_c = []
        for mi in range(8):
            ps = T(pB, [P, 1], F32, tag="bps")
            for c in range(4):
                nc.tensor.matmul(ps[:], pinT[c][:, mi * P:(mi + 1) * P],
                                 ln1b_cb[c][:], start=(c == 0), stop=(c == 3))
            t = T(cst, [P, 1], F32, tag=f"pinbf{mi}")
            nc.vector.tensor_add(t[:], ps[:], pinb_raw[mi][:])
            pinb_c.append(t)
        pB.release()
        for c in range(4):
            nc.vector.tensor_scalar_mul(pinT[c][:], pinT[c][:], ln1w_c[c][:])

        # ---- gnconv: pin + depthwise conv + gate chain ----
        # fused-channel chunks of 128: q=0 holds pwa(0:32)+dw0(32:64)+dw1(64:128)
        gcv = tc.alloc_tile_pool(name="gcv", bufs=1, side="right")
        dg = tc.alloc_tile_pool(name="dg", bufs=1, side="right")
        pM = tc.alloc_tile_pool(name="pM", bufs=2, space="PSUM")
        pDW = tc.alloc_tile_pool(name="pDW", bufs=4, space="PSUM")

        pwa = T(gcv, [32, BC, 32, 32], BF16, tag="pwa")
        # padded pin-output tiles, cycled q%2; margins zeroed ONCE
        abc_slots = []
        for s in range(2):
            t = T(gcv, [P, BC, 38, 38], BF16, tag=f"abcs{s}")
            nc.vector.memset(t[:], 0.0)
            abc_slots.append(t)
        abc_t = [None] * 8
        # shifted-replica tiles: T_slots[s][g][32i+c, b, h, w] =
        #   abc_slot[s][32g+c, b, h+i, w]  (row-shift i baked into partitions)
        T_slots = [[T(dg, [P, BC, 38, 38], BF16, tag=f"Ts{s}_{g}")
                    for g in range(4)] for s in range(2)]

        def pin_chunk(q):
            abc = abc_slots[q % 2]
            abc_t[q] = abc
            for (b, ht) in NB:
                r0 = ht * 16
                ps = T(pM, [P, 16, 32], F32, tag="pinps")
                for c in range(4):
                    nc.tensor.matmul(ps[:], pinT[c][:, q * P:(q + 1) * P],
                                     xn[c][:, b, r0:r0 + 16, :],
                                     start=(c == 0), stop=(c == 3))
                nc.scalar.activation(abc[:, b, 3 + r0:3 + r0 + 16, 3:35],
                                     ps[:], AF.Identity, bias=pinb_c[q][:])
                if q == 0:
                    nc.scalar.activation(pwa[:, b, r0:r0 + 16, :], ps[:32],
                                         AF.Identity, bias=pinb_c[0][:32])
            # build the 4 shifted-replica tiles for this chunk: one flat
            # contiguous run per partition (shift baked into the src offset;
            # the cross-image bleed rows are never read by the dw matmuls).
            # Issue is split across both hardware DGEs (sync + scalar) so
            # neither sequencer serializes behind this stream.
            abc_f = abc.rearrange("p b h w -> p (b h w)")
            for g in range(4):
                if q == 0 and g == 0:
                    continue
                tg_f = T_slots[q % 2][g].rearrange("p b h w -> p (b h w)")
                for i in range(4):
                    n = BC * 38 * 38 - 38 * i
                    nc.scalar.dma_start(tg_f[32 * i:32 * (i + 1), 0:n],
                                        abc_f[32 * g:32 * (g + 1), 38 * i:38 * i + n])

        def dw_stationaries(q):
            # scaled 4/3-strip diag32 stationaries, one per (g, band, dx)
            sA = [[None] * 7 for _ in range(4)]
            sB = [[None] * 7 for _ in range(4)]
            for g in range(4):
                if q == 0 and g == 0:
                    continue
                for dx in range(7):
                    ta = T(dg, [P, 32], BF16, tag=f"sA{g}_{dx}", bufs=2)
                    nc.vector.tensor_scalar_mul(ta[:], i4[:],
                                                wcolA[q][g][:, dx:dx + 1])
                    sA[g][dx] = ta
                    tb = T(dg, [96, 32], BF16, tag=f"sB{g}_{dx}", bufs=2)
                    nc.vector.tensor_scalar_mul(tb[:], i4[0:96, :],
                                                wcolB[q][g][:, dx:dx + 1])
                    sB[g][dx] = tb
            return sA, sB

        y3 = [None, None]
        y4 = [None] * 4
        pin_chunk(0)
        sAB = [None] * 8
        sAB[0] = dw_stationaries(0)
        for q in range(8):
            if q + 1 < 8:
                pin_chunk(q + 1)
                sAB[q + 1] = dw_stationaries(q + 1)
            sA, sB = sAB[q]
            Tg = T_slots[q % 2]
            dwt = T(gcv, [P, BC, 32, 32], BF16, tag="dwt", bufs=2)
            if q == 0:
                dw0t = T(gcv, [32, BC, 32, 32], BF16, tag="dw0t")
                dw1t = T(gcv, [64, BC, 32, 32], BF16, tag="dw1t")
            groups = [1, 2, 3] if q == 0 else [0, 1, 2, 3]
            for (b, ht) in NB:
                r0 = ht * 16
                cps = T(pDW, [P, 16, 32], F32, tag="convps")
                for dx in range(7):
                    for g in groups:
                        nc.tensor.matmul(
                            cps[32 * g:32 * (g + 1), :, :], sA[g][dx][:],
                            Tg[g][:, b, r0:r0 + 16, dx:dx + 32],
                            start=(dx == 0), stop=False,
                            skip_group_check=True,
                            tile_position=(0, 32 * g))
                for dx in range(7):
                    for g in groups:
                        nc.tensor.matmul(
                            cps[32 * g:32 * (g + 1), :, :], sB[g][dx][:],
                            Tg[g][0:96, b, r0 + 4:r0 + 20, dx:dx + 32],
                            start=False, stop=(dx == 6),
                            skip_group_check=True,
                            tile_position=(0, 32 * g))
                if q == 0:
                    nc.vector.tensor_scalar_add(dw0t[:32, b, r0:r0 + 16, :],
                                                cps[32:64], dwb00[:])
                    nc.vector.tensor_scalar_add(dw1t[:64, b, r0:r0 + 16, :],
                                                cps[64:128], dwb01[:])
                else:
                    nc.vector.tensor_scalar_add(dwt[:, b, r0:r0 + 16, :],
                                                cps[:], dwb_c[q][:])

            # gate chain, consuming dwt immediately
            if q == 0:
                y0 = T(gcv, [P, BC, 32, 32], BF16, tag="ya")
                nc.vector.memset(y0[:], 0.0)
                nc.vector.tensor_mul(y0[:32], pwa[:32], dw0t[:32])
                y1 = T(gcv, [P, BC, 32, 32], BF16, tag="yb")
                nc.vector.memset(y1[:], 0.0)
                for (b, ht) in NB:
                    r0 = ht * 16
                    ps = T(pM, [P, 16, 32], F32, tag="mmps")
                    nc.tensor.matmul(ps[:64], pw0T[:], y0[:, b, r0:r0 + 16, :],
                                     start=True, stop=True)
                    nc.vector.scalar_tensor_tensor(
                        y1[:64, b, r0:r0 + 16, :], ps[:64], pwb_c[0][0][:],
                        dw1t[:64, b, r0:r0 + 16, :], op0=AL.add, op1=AL.mult)
            elif q == 1:
                y2 = T(gcv, [P, BC, 32, 32], BF16, tag="ya")
                for (b, ht) in NB:
                    r0 = ht * 16
                    ps = T(pM, [P, 16, 32], F32, tag="mmps")
                    nc.tensor.matmul(ps[:], pw1T[:], y1[:, b, r0:r0 + 16, :],
                                     start=True, stop=True)
                    nc.vector.scalar_tensor_tensor(
                        y2[:, b, r0:r0 + 16, :], ps[:], pwb_c[1][0][:],
                        dwt[:, b, r0:r0 + 16, :], op0=AL.add, op1=AL.mult)
            elif q in (2, 3):
                k = q - 2
                y3[k] = T(gcv, [P, BC, 32, 32], BF16, tag=f"y3_{k}")
                for (b, ht) in NB:
                    r0 = ht * 16
                    ps = T(pM, [P, 16, 32], F32, tag="mmps")
                    nc.tensor.matmul(ps[:], pw2T[:, k * P:(k + 1) * P],
                                     y2[:, b, r0:r0 + 16, :],
                                     start=True, stop=True)
                    nc.vector.scalar_tensor_tensor(
                        y3[k][:, b, r0:r0 + 16, :], ps[:], pwb_c[2][k][:],
                        dwt[:, b, r0:r0 + 16, :], op0=AL.add, op1=AL.mult)
            else:
                k = q - 4
                y4[k] = T(gcv, [P, BC, 32, 32], BF16, tag=f"y4_{k}")
                for (b, ht) in NB:
                    r0 = ht * 16
                    ps = T(pM, [P, 16, 32], F32, tag="mmps")
                    for j in range(2):
                        nc.tensor.matmul(ps[:], pw3T[j][:, k * P:(k + 1) * P],
                                         y3[j][:, b, r0:r0 + 16, :],
                                         start=(j == 0), stop=(j == 1))
                    nc.vector.scalar_tensor_tensor(
                        y4[k][:, b, r0:r0 + 16, :], ps[:], pwb_c[3][k][:],
                        dwt[:, b, r0:r0 + 16, :], op0=AL.add, op1=AL.mult)
        dg.release()
        pDW.release()
        x2p = tc.alloc_tile_pool(name="x2p", bufs=1, side="left")
        x2_t = []
        for c in range(4):
            t = T(x2p, [P, BC, 32, 32], F32, tag=f"x2{c}")
            dma(t[:], x_cf[c * P:(c + 1) * P])
            x2_t.append(t)

        # ---- fc1 bias fold with ln2_b, then scale fc1T by ln2_w ----
        pB2 = tc.alloc_tile_pool(name="pB2", bufs=2, space="PSUM")
        fc1b_c = []
        for q in range(16):
            ps = T(pB2, [P, 1], F32, tag="bps2")
            for c in range(4):
                nc.tensor.matmul(ps[:], fc1T[c][:, q * P:(q + 1) * P],
                                 ln2b_cb[c][:], start=(c == 0), stop=(c == 3))
            raw = col(fc1_b, q * P, P, f"fc1b{q}")
            t = T(cst, [P, 1], F32, tag=f"fc1bf{q}")
            nc.vector.tensor_add(t[:], ps[:], raw[:])
            ts = T(cst, [P, 1], F32, tag=f"fc1bs{q}")
            nc.vector.tensor_scalar_mul(ts[:], t[:], 1.702)
            fc1b_c.append((t, ts))
        pB2.release()
        for c in range(4):
            nc.vector.tensor_scalar_mul(fc1T[c][:], fc1T[c][:], ln2w_c[c][:])

        # ---- proj_out + residual: x2 = x + g1*(pout@y4 + pout_b) ----
        for co in range(4):
            for (b, ht) in NB:
                r0 = ht * 16
                ps = T(pM, [P, 16, 32], F32, tag="mmps")
                for j in range(4):
                    nc.tensor.matmul(ps[:], poutT[j][:, co * P:(co + 1) * P],
                                     y4[j][:, b, r0:r0 + 16, :],
                                     start=(j == 0), stop=(j == 3))
                tres = T(tp, [P, 16, 32], BF16, tag="tres")
                nc.scalar.activation(tres[:], ps[:], AF.Identity,
                                     bias=bg1_c[co][:], scale=g1_c[co][:])
                nc.vector.tensor_add(x2_t[co][:, b, r0:r0 + 16, :],
                                     x2_t[co][:, b, r0:r0 + 16, :], tres[:])
        pM.release()
        gcv.release()
        xnp.release()

        xnp2 = tc.alloc_tile_pool(name="xnp2", bufs=1, side="right")

        # ---- LN2 -> xn2 ----
        lnp2 = tc.alloc_tile_pool(name="lnp2", bufs=1, side="right")
        pS2 = tc.alloc_tile_pool(name="pS2", bufs=1, space="PSUM")
        u2b, r2b, xbf2 = layernorm_bcast(lnp2, pS2, x2_t)
        xn2 = []
        for c in range(4):
            t = T(xnp2, [P, BC, 32, 32], BF16, tag=f"xn2{c}")
            nc.vector.tensor_sub(t[:], xbf2[c][:], u2b[:])
            nc.vector.tensor_mul(t[:], t[:], r2b[:])
            xn2.append(t)
        pS2.release()
        lnp2.release()
        # ---- MLP: fc1 -> gelu -> fc2, fused final residual ----
        pF = tc.alloc_tile_pool(name="pF", bufs=1, space="PSUM")
        pH = tc.alloc_tile_pool(name="pH", bufs=2, space="PSUM")
        for (b, ht) in NB:
            r0 = ht * 16
            fps = [T(pF, [P, 16, 32], F32, tag=f"fco{i}") for i in range(4)]
            for q in range(16):
                hps = T(pH, [P, 512], F32, tag="hps")
                for c in range(4):
                    nc.tensor.matmul(hps[:], fc1T[c][:, q * P:(q + 1) * P],
                                     xn2[c][:, b, r0:r0 + 16, :],
                                     start=(c == 0), stop=(c == 3))
                # gelu(z) ~= z * sigmoid(1.702 z), z = hps + fc1_b'
                sig = T(tp, [P, 512], BF16, tag="sig", bufs=2)
                nc.scalar.activation(sig[:], hps[:], AF.Sigmoid,
                                     bias=fc1b_c[q][1][:], scale=1.702)
                h = T(tp, [P, 512], BF16, tag="h", bufs=2)
                nc.vector.scalar_tensor_tensor(h[:], hps[:], fc1b_c[q][0][:],
                                               sig[:], op0=AL.add, op1=AL.mult)
                for co in range(4):
                    nc.tensor.matmul(fps[co][:], fc2T[q][:, co * P:(co + 1) * P],
                                     h[:], start=(q == 0), stop=(q == 15),
                                     skip_group_check=True)
            for co in range(4):
                t2 = T(tp, [P, 16, 32], BF16, tag="t2")
                nc.vector.tensor_scalar(t2[:], fps[co][:], g2_c[co][:],
                                        bg2_c[co][:], op0=AL.mult, op1=AL.add)
                ot = T(tp, [P, 16, 32], F32, tag="ot")
                nc.vector.tensor_add(ot[:], x2_t[co][:, b, r0:r0 + 16, :], t2[:])
                dma(out_cf[co * P:(co + 1) * P, b, r0:r0 + 16, :], ot[:])
        pH.release()
        pF.release()

        xnp2.release()
        x2p.release()
        dwp.release()
        drp.release()
        wPw.release()
        wPin.release()
        tp.release()
        cst.release()

    nc.compile()
    return nc


def kernel(**inputs):
    from concourse import bass_utils

    if "nc" not in _CACHE:
        _CACHE["nc"] = _build()
    nc = _CACHE["nc"]

    x = np.ascontiguousarray(inputs["x"], dtype=np.float32)
    weights = {k: np.ascontiguousarray(np.asarray(v), dtype=np.float32)
               for k, v in inputs.items() if k != "x"}
    in_maps = []
    for i in range(NCORES):
        m = dict(weights)
        m["x"] = x[i * BC:(i + 1) * BC]
        in_maps.append(m)
    res = bass_utils.run_bass_kernel_spmd(nc, in_maps,
                                          core_ids=list(range(NCORES)))
    out = np.concatenate([res.results[i]["out"] for i in range(NCORES)], axis=0)
    return out.astype(np.float32)
